# revision 21
# baseline (speedup 1.0000x reference)
"""MultiHeadedAttention Trainium2 Bass kernel (v6).

Full inputs in, full output out. 8 cores = 4 batches x 2 head-pairs.

Per-core structure (bf16 matmuls except the fp8-DoubleRow x-accumulation):
  - 512-col matmuls cost ~375 ns regardless of dtype (overhead-bound), so
    the q/k/score chain stays bf16 for precision; fp8 pays off only where
    it halves the INSTRUCTION count: the x-accumulation pairs two m-blocks
    per DoubleRow matmul (pt [128, 2, 1024] fp8 planes, vt [128,4,2,72] fp8
    with a 16B-aligned k-tile stride and a ones column at index 64 for the
    softmax denominator).
  - Projections bf16 (2 matmuls per 512-wide window); k evac ACT, q evac
    DVE (+bq); V^T per window: 8 matmuls into one PSUM tile, ONE evac op
    rearranged into the vt layout (engine alternates per window).
  - Exp: ACT (exp(sc/8) -> fp8) for g%16 in {0,2,..,14,15}; DVE
    Schraudolph-to-e4m3 (int8(trunc(sc*log2e + 56.156)) bitcast) for odd
    g%16 except 15 -> 36 ACT / 28 DVE, and chunk-boundary pairs land fully
    on ACT so the DVE queue is drained when a chunk tail needs it.
  - Chunks 0-2 normalize via the DMA-bounce reciprocal broadcast (latency
    hidden); the final chunk skips on-device normalization entirely: it
    emits per-head UNNORMALIZED out-projection partials (same 4 matmuls,
    just not h-accumulated) plus the denominator rows, and the host divides
    and sums -- no reciprocal/broadcast chain in the critical tail.
  - Out-projection bf16; both oc-blocks accumulate into one [128, 1024]
    PSUM tile -> ONE evac -> fp16 -> one DMA per chunk.
Host sums the two per-batch partials (fp16) and adds bm + wm @ bv in fp32.
"""

import sys

if "/opt/trn_rl_repo" not in sys.path:
    sys.path.insert(0, "/opt/trn_rl_repo")

import numpy as np
import ml_dtypes

BF = ml_dtypes.bfloat16
F8 = ml_dtypes.float8_e4m3

B, D, N, H = 4, 256, 2048, 4
DIM = D // H  # 64
NW = 4  # 512-wide input windows
MB = 16  # 128-wide m blocks per chunk
NC = 4  # 512-wide n chunks
G = NC * MB  # 64 iterations
NP = G // 2  # 32 pairs

ACT_SCALE = 1.0 / 8.0  # sc = s_true; pt = exp(sc/8)
C_SCH = 0.0430
S8 = float(np.log2(np.e))
# bits = trunc(sc*S8 + B8); exp(s/8) stays ~8 sigma from both the e4m3 Inf
# boundary (s > 44) and negative-bits (s < -39)
B8 = float(8.0 * (7.0 - C_SCH) + 0.5)  # +0.5: truncation -> round

_CACHE = {}


def _emit(ctx, tc, io):
    import concourse.bass as bass
    import concourse.mybir as mybir

    nc = tc.nc
    f32 = mybir.dt.float32
    bf16 = mybir.dt.bfloat16
    fp16 = mybir.dt.float16
    fp8 = mybir.dt.float8e4
    i8 = mybir.dt.int8
    EXP = mybir.ActivationFunctionType.Exp
    MUL = mybir.AluOpType.mult
    ADD = mybir.AluOpType.add
    DR = mybir.MatmulPerfMode.DoubleRow

    const = ctx.enter_context(tc.tile_pool(name="const", bufs=1))
    xin = ctx.enter_context(tc.tile_pool(name="xin", bufs=1))
    kqp = ctx.enter_context(tc.tile_pool(name="kqp", bufs=4))
    vtp = ctx.enter_context(tc.tile_pool(name="vtp", bufs=4))
    ptp = ctx.enter_context(tc.tile_pool(name="ptp", bufs=3))
    pxe_p = ctx.enter_context(tc.tile_pool(name="pxe", bufs=4))
    xhp = ctx.enter_context(tc.tile_pool(name="xhp", bufs=8))
    work = ctx.enter_context(tc.tile_pool(name="work", bufs=2))
    outp = ctx.enter_context(tc.tile_pool(name="outp", bufs=3))
    psA = ctx.enter_context(tc.tile_pool(name="psA", bufs=3, space="PSUM"))
    psX = ctx.enter_context(tc.tile_pool(name="psX", bufs=2, space="PSUM"))
    dpool = ctx.enter_context(tc.tile_pool(name="dpool", bufs=2, space="DRAM"))

    # ---- constants (gpsimd memsets run during the engine-preamble window)
    wu_a = const.tile([128, 128], bf16, tag="wu_a")
    nc.gpsimd.memset(wu_a, 0.0)
    wu_b = const.tile([128, 512], bf16, tag="wu_b")
    nc.gpsimd.memset(wu_b, 0.0)
    junk = const.tile([128, 2], f32, tag="junk")

    # ---- weights + xq on sync; xk/xv on scalar (xv w0 on gpsimd)
    x_sb = {}
    for name in ("xq", "xk", "xv"):
        x_sb[name] = xin.tile([128, 2, 2048], bf16, tag=name, name=name)
    srcs = {n: io[n].rearrange("(c p) n -> p c n", p=128) for n in ("xq", "xk", "xv")}

    wqkv = const.tile([128, 2, 384], bf16, tag="wqkv")
    nc.sync.dma_start(wqkv, io["wqkv"].rearrange("(c p) o -> p c o", p=128))
    nc.sync.dma_start(x_sb["xk"][:, 1:2, 0:512], srcs["xk"][:, 1:2, 0:512])
    nc.sync.dma_start(x_sb["xq"][:, 1:2, 0:512], srcs["xq"][:, 1:2, 0:512])
    bq_sb = const.tile([128, 1], f32, tag="bq")
    nc.sync.dma_start(bq_sb, io["bq"])
    wm_sb = const.tile([64, 2, 256], bf16, tag="wm")
    nc.sync.dma_start(wm_sb, io["wm"].rearrange("(t o) c -> o t c", t=2))
    for w in range(1, NW):
        nc.sync.dma_start(
            x_sb["xq"][:, :, 512 * w : 512 * (w + 1)],
            srcs["xq"][:, :, 512 * w : 512 * (w + 1)],
        )

    nc.scalar.dma_start(x_sb["xk"][:, 0:1, 0:512], srcs["xk"][:, 0:1, 0:512])
    nc.scalar.dma_start(x_sb["xq"][:, 0:1, 0:512], srcs["xq"][:, 0:1, 0:512])

    # ---- k/q bf16; vt fp8 with ones column
    k_w, q_w, vt_w = [], [], []
    for w in range(NW):
        k_w.append(kqp.tile([128, 512], bf16, tag="kw", name=f"kw{w}"))
        q_w.append(kqp.tile([128, 512], bf16, tag="qw", name=f"qw{w}"))
        vt = vtp.tile([128, 4, 2, 72], fp8, tag="vt", name=f"vt{w}")
        nc.gpsimd.memset(vt[:, :, :, 64:65], 1.0)
        vt_w.append(vt)

    nc.gpsimd.dma_start(x_sb["xv"][:, :, 0:512], srcs["xv"][:, :, 0:512])

    # PE warmup across the input-DMA ramp (HAM clock gate release)
    wu_ps = psA.tile([128, 1024], f32, tag="ps", name="wu_ps")
    for _ in range(10):
        nc.tensor.matmul(wu_ps[:, 0:512], lhsT=wu_a, rhs=wu_b, start=True, stop=True)

    # ACT table trigger, then remaining k/v windows interleaved in need-order
    nc.scalar.activation(junk[:, 0:1], wu_a[:, 0:1], EXP)
    for w in range(1, NW):
        nc.scalar.dma_start(
            x_sb["xk"][:, :, 512 * w : 512 * (w + 1)],
            srcs["xk"][:, :, 512 * w : 512 * (w + 1)],
        )
        nc.scalar.dma_start(
            x_sb["xv"][:, :, 512 * w : 512 * (w + 1)],
            srcs["xv"][:, :, 512 * w : 512 * (w + 1)],
        )

    # ---- projection emitters ----
    def proj_k(w):
        ps = psA.tile([128, 1024], f32, tag="ps", name=f"psk{w}")
        for c2 in range(2):
            nc.tensor.matmul(
                ps[:, 0:512],
                lhsT=wqkv[:, c2, 128:256],
                rhs=x_sb["xk"][:, c2, 512 * w : 512 * (w + 1)],
                start=(c2 == 0),
                stop=(c2 == 1),
            )
        if w == 0:
            nc.vector.tensor_copy(k_w[w], ps[:, 0:512])
        else:
            nc.scalar.copy(k_w[w], ps[:, 0:512])

    def proj_q(c):
        ps = psA.tile([128, 1024], f32, tag="ps", name=f"psq{c}")
        for c2 in range(2):
            nc.tensor.matmul(
                ps[:, 0:512],
                lhsT=wqkv[:, c2, 0:128],
                rhs=x_sb["xq"][:, c2, 512 * c : 512 * (c + 1)],
                start=(c2 == 0),
                stop=(c2 == 1),
            )
        nc.vector.tensor_scalar_add(q_w[c], ps[:, 0:512], bq_sb)

    def vt_block(w):
        # 8 matmuls into one PSUM tile (cols off*128 + h*64 + d), ONE evac
        vt = vt_w[w]
        ps = psA.tile([128, 1024], f32, tag="ps", name=f"psvt{w}")
        for off in range(4):
            ms = slice(512 * w + 128 * off, 512 * w + 128 * (off + 1))
            pvt = ps[:, 128 * off : 128 * (off + 1)]
            for c2 in range(2):
                nc.tensor.matmul(
                    pvt,
                    lhsT=x_sb["xv"][:, c2, ms],
                    rhs=wqkv[:, c2, 256:384],
                    start=(c2 == 0),
                    stop=(c2 == 1),
                )
        dst = vt[:, :, :, 0:64]
        src = ps[:, 0:512].rearrange("m (o h d) -> m o h d", o=4, h=2)
        if w % 2 == 0:
            nc.scalar.copy(dst, src)
        else:
            nc.vector.tensor_copy(dst, src)

    # ---- software-pipelined attention ----
    pt_t, px_t, xh_t = {}, {}, {}

    def emit_sc(g):
        c, mb = divmod(g, MB)
        w, off = divmod(mb, 4)
        msl = slice(off * 128, (off + 1) * 128)
        sc = psA.tile([128, 1024], f32, tag="ps", name=f"sc{g}")
        for h in range(2):
            nc.tensor.matmul(
                sc[:, 512 * h : 512 * (h + 1)],
                lhsT=k_w[w][64 * h : 64 * (h + 1), msl],
                rhs=q_w[c][64 * h : 64 * (h + 1), :],
                start=True,
                stop=True,
                tile_position=(64 * h, 0),
            )
        p, i = divmod(g, 2)
        if i == 0:
            pt = ptp.tile([128, 2, 1024], fp8, tag="pt", name=f"pt{p}")
            pt_t[p] = pt
        else:
            pt = pt_t[p]
        if g % 16 in (1, 3, 5, 7, 9, 11, 13) or g == G - 1:
            nc.vector.tensor_scalar(pt[:, i, :].bitcast(i8), sc, S8, B8, MUL, ADD)
        else:
            nc.scalar.activation(pt[:, i, :], sc, EXP, scale=ACT_SCALE)

    def emit_xdr(p):
        c, j = divmod(p, 8)
        w = j // 2
        o2 = (2 * j) % 4
        if j == 0:
            px_t[c] = [
                psX.tile([65, 512], f32, tag="px", name=f"px{c}_{h}") for h in range(2)
            ]
        pt = pt_t.pop(p)
        for h in range(2):
            nc.tensor.matmul(
                px_t[c][h],
                lhsT=vt_w[w][:, o2 : o2 + 2, h, 0:65],
                rhs=pt[:, :, 512 * h : 512 * (h + 1)],
                start=(j == 0),
                stop=(j == 7),
                perf_mode=DR,
                skip_group_check=True,
            )

    def emit_evacs(c):
        px = px_t.pop(c)
        pxe = []
        for h in range(2):
            e = pxe_p.tile([65, 512], f32, tag="pxe", name=f"pxe{c}_{h}")
            if h == 0:
                nc.scalar.copy(e, px[h])
            else:
                nc.vector.tensor_copy(e, px[h])
            pxe.append(e)
        return pxe

    def chunk_tail_rest(c, pxe):
        # 1/sums via [128, 8] reshape, DRAM bounce, partition-broadcast read
        s128 = work.tile([128, 8], f32, tag="s128", name=f"s128_{c}")
        for h in range(2):
            nc.sync.dma_start(s128[64 * h : 64 * (h + 1), :], pxe[h][64:65, :])
        r128 = work.tile([128, 8], f32, tag="r128", name=f"r128_{c}")
        nc.vector.reciprocal(r128, s128)
        r_dram = dpool.tile([1, 1024], f32, tag="r_dram", name=f"r_dram{c}")
        nc.sync.dma_start(r_dram.rearrange("1 (p f) -> p f", p=128), r128)
        r_bc = work.tile([64, 2, 512], f32, tag="r_bc", name=f"r_bc{c}")
        for h in range(2):
            r_src = bass.AP(
                tensor=r_dram.tensor,
                offset=r_dram.offset + h * 512,
                ap=[[0, 64], [1, 512]],
            )
            nc.sync.dma_start(r_bc[:, h, :], r_src)
        for h in range(2):
            xh = xhp.tile([64, 512], bf16, tag="xh", name=f"xh{c}_{h}")
            nc.gpsimd.tensor_mul(xh, pxe[h][0:64, :], r_bc[:, h, :])
            xh_t[(c, h)] = xh

    def final_tail(px):
        # last chunk: per-head UNNORMALIZED out-projection partials + the
        # denominator rows go to DRAM; the host divides and sums. Removes the
        # whole reciprocal/broadcast chain from the critical tail.
        xu = []
        for h in range(2):
            e = xhp.tile([65, 512], fp16, tag="xu", name=f"xu3_{h}")
            if h == 0:
                nc.scalar.copy(e, px[h])
            else:
                nc.vector.tensor_copy(e, px[h])
            xu.append(e)
        ot3 = [
            outp.tile([128, 2, 512], fp16, tag="ot3", name=f"ot3_{h}") for h in range(2)
        ]
        for h in range(2):
            po = psA.tile([128, 1024], f32, tag="ps", name=f"po3_{h}")
            for oc in range(2):
                nc.tensor.matmul(
                    po[:, 512 * oc : 512 * (oc + 1)],
                    lhsT=wm_sb[:, h, 128 * oc : 128 * (oc + 1)],
                    rhs=xu[h][0:64, :],
                    start=True,
                    stop=True,
                )
            src_ = po.rearrange("p (t n) -> p t n", t=2)
            if h == 0:
                nc.scalar.copy(ot3[h], src_)
            else:
                nc.vector.tensor_copy(ot3[h], src_)
        o3 = io["out3"].rearrange("h (t p) n -> h p t n", p=128)
        nc.sync.dma_start(o3[0], ot3[0])
        nc.gpsimd.dma_start(o3[1], ot3[1])
        nc.sync.dma_start(io["den3"][0:1, :], xu[0][64:65, :])
        nc.gpsimd.dma_start(io["den3"][1:2, :], xu[1][64:65, :])

    def out_proj(c):
        ot = outp.tile([128, 2, 512], fp16, tag="ot", name=f"ot{c}")
        po = psA.tile([128, 1024], f32, tag="ps", name=f"po{c}")
        for oc in range(2):
            ocs = slice(128 * oc, 128 * (oc + 1))
            dst = po[:, 512 * oc : 512 * (oc + 1)]
            nc.tensor.matmul(dst, lhsT=wm_sb[:, 0, ocs], rhs=xh_t[(c, 0)], start=True, stop=False)
            nc.tensor.matmul(dst, lhsT=wm_sb[:, 1, ocs], rhs=xh_t[(c, 1)], start=False, stop=True)
        src = po.rearrange("p (t n) -> p t n", t=2)
        if c % 2 == 0:
            nc.scalar.copy(ot, src)
        else:
            nc.vector.tensor_copy(ot, src)
        dst_dram = io["out"].rearrange("(t p) n -> p t n", p=128)[:, :, 512 * c : 512 * (c + 1)]
        eng = nc.sync if c % 2 == 0 else nc.gpsimd
        eng.dma_start(dst_dram, ot)

    def maybe_proj(g):
        c, mb = divmod(g, MB)
        if c == 0:
            if mb in (4, 8, 12):
                proj_k(mb // 4)
            elif mb in (6, 10, 14):
                vt_block((mb - 2) // 4)
        if mb == 0 and c in (1, 2, 3):
            proj_q(c)

    # prelude: window 0 of everything, then pair 0 of scores/exp
    proj_k(0)
    proj_q(0)
    vt_block(0)
    emit_sc(0)
    emit_sc(1)

    for p in range(NP):
        ga, gb = 2 * p + 2, 2 * p + 3
        if p % 8 != 7:
            maybe_proj(ga)
            emit_sc(ga)
            maybe_proj(gb)
            emit_sc(gb)
            emit_xdr(p)
            if p % 8 == 6 and p // 8 >= 1:
                out_proj(p // 8 - 1)
        else:
            c = p // 8
            if ga < G:
                maybe_proj(ga)
                emit_sc(ga)
                emit_xdr(p)
                pxe = emit_evacs(c)
                maybe_proj(gb)
                emit_sc(gb)
                chunk_tail_rest(c, pxe)
            else:
                emit_xdr(p)
                final_tail(px_t.pop(c))


def _build_nc():
    key = "nc"
    if key in _CACHE:
        return _CACHE[key]
    from contextlib import ExitStack

    import concourse.mybir as mybir
    import concourse.tile as tile
    from concourse import bacc

    f32 = mybir.dt.float32
    bf16 = mybir.dt.bfloat16
    fp16 = mybir.dt.float16
    nc = bacc.Bacc("TRN2", target_bir_lowering=False, debug=False, num_devices=8)
    io = {}
    for name, shape, dt_ in (
        ("xq", [256, 2048], bf16),
        ("xk", [256, 2048], bf16),
        ("xv", [256, 2048], bf16),
        ("wqkv", [256, 384], bf16),
        ("bq", [128, 1], f32),
        ("wm", [128, 256], bf16),
    ):
        io[name] = nc.dram_tensor(name, shape, dt_, kind="ExternalInput").ap()
    io["out"] = nc.dram_tensor("out", [256, 2048], fp16, kind="ExternalOutput").ap()
    io["out3"] = nc.dram_tensor("out3", [2, 256, 512], fp16, kind="ExternalOutput").ap()
    io["den3"] = nc.dram_tensor("den3", [2, 512], fp16, kind="ExternalOutput").ap()

    with tile.TileContext(nc) as tc:
        with ExitStack() as ctx:
            _emit(ctx, tc, io)
    nc.compile()
    _CACHE[key] = nc
    return nc


def make_in_maps(query, key, value, wq, bq, wk, bk, wv, bv, wm, bm):
    f = lambda a: np.ascontiguousarray(np.asarray(a), dtype=np.float32)
    fb = lambda a: np.ascontiguousarray(np.asarray(a, dtype=np.float32)).astype(BF)
    query, key, value = f(query), f(key), f(value)
    wq, wk, wv, wm = f(wq), f(wk), f(wv), f(wm)
    bq = f(bq)
    in_maps = []
    for core in range(8):
        b, pair = divmod(core, 2)
        hs = (2 * pair, 2 * pair + 1)
        idx = np.array([d * H + h for h in hs for d in range(DIM)])
        wqkv8 = np.concatenate([wq[idx].T, wk[idx].T, wv[idx].T], axis=1)
        wmh = np.concatenate([wm[:, idx[:64]].T, wm[:, idx[64:]].T], axis=0)
        m = {
            "xq": fb(query[b]),
            "xk": fb(key[b]),
            "xv": fb(value[b]),
            "wqkv": fb(wqkv8),
            "bq": f(bq[idx].reshape(128, 1)),
            "wm": fb(wmh),
        }
        in_maps.append(m)
    return in_maps


def run(in_maps, trace=False, **kw):
    from concourse import bass_utils

    nc = _build_nc()
    return bass_utils.run_bass_kernel_spmd(
        nc, in_maps, core_ids=list(range(8)), trace=trace, **kw
    )


def gather(results, wm, bv, bm):
    wm = np.asarray(wm, dtype=np.float32)
    bv = np.asarray(bv, dtype=np.float32)
    bm = np.asarray(bm, dtype=np.float32)
    corr = bm + wm @ bv
    outs = []
    for r in results:
        o = np.asarray(r["out"], dtype=np.float32)
        o3 = np.asarray(r["out3"], dtype=np.float32)
        den = np.asarray(r["den3"], dtype=np.float32)
        o[:, 3 * 512 :] = (o3 / den[:, None, :]).sum(axis=0)
        outs.append(o)
    return np.stack([outs[2 * b] + outs[2 * b + 1] + corr[:, None] for b in range(B)])


def kernel(query, key, value, wq, bq, wk, bk, wv, bv, wm, bm):
    in_maps = make_in_maps(query, key, value, wq, bq, wk, bk, wv, bv, wm, bm)
    res = run(in_maps)
    return gather(res.results, wm, bv, bm)


# revision 22
# speedup vs baseline: 1.0499x; 1.0499x over previous
"""MultiHeadedAttention Trainium2 Bass kernel (v6).

Full inputs in, full output out. 8 cores = 4 batches x 2 head-pairs.

Per-core structure (bf16 matmuls except the fp8-DoubleRow x-accumulation):
  - 512-col matmuls cost ~375 ns regardless of dtype (overhead-bound), so
    the q/k/score chain stays bf16 for precision; fp8 pays off only where
    it halves the INSTRUCTION count: the x-accumulation pairs two m-blocks
    per DoubleRow matmul (pt [128, 2, 1024] fp8 planes, vt [128,4,2,72] fp8
    with a 16B-aligned k-tile stride and a ones column at index 64 for the
    softmax denominator).
  - Projections bf16 (2 matmuls per 512-wide window); k evac ACT, q evac
    DVE (+bq); V^T per window: 8 matmuls into one PSUM tile, ONE evac op
    rearranged into the vt layout (engine alternates per window).
  - Exp: ACT (exp(sc/8) -> fp8) for g%16 in {0,2,..,14,15}; DVE
    Schraudolph-to-e4m3 (int8(trunc(sc*log2e + 56.156)) bitcast) for odd
    g%16 except 15 -> 36 ACT / 28 DVE, and chunk-boundary pairs land fully
    on ACT so the DVE queue is drained when a chunk tail needs it.
  - Chunks 0-2 normalize via the DMA-bounce reciprocal broadcast (latency
    hidden); the final chunk skips on-device normalization entirely: it
    emits per-head UNNORMALIZED out-projection partials (same 4 matmuls,
    just not h-accumulated) plus the denominator rows, and the host divides
    and sums -- no reciprocal/broadcast chain in the critical tail.
  - Out-projection bf16; both oc-blocks accumulate into one [128, 1024]
    PSUM tile -> ONE evac -> fp16 -> one DMA per chunk.
Host sums the two per-batch partials (fp16) and adds bm + wm @ bv in fp32.
"""

import sys

if "/opt/trn_rl_repo" not in sys.path:
    sys.path.insert(0, "/opt/trn_rl_repo")

import numpy as np
import ml_dtypes

BF = ml_dtypes.bfloat16
F8 = ml_dtypes.float8_e4m3

B, D, N, H = 4, 256, 2048, 4
DIM = D // H  # 64
NW = 4  # 512-wide input windows
MB = 16  # 128-wide m blocks per chunk
NC = 4  # 512-wide n chunks
G = NC * MB  # 64 iterations
NP = G // 2  # 32 pairs

ACT_SCALE = 1.0 / 8.0  # sc = s_true; pt = exp(sc/8)
C_SCH = 0.0430
S8 = float(np.log2(np.e))
# bits = trunc(sc*S8 + B8); exp(s/8) stays ~8 sigma from both the e4m3 Inf
# boundary (s > 44) and negative-bits (s < -39)
B8 = float(8.0 * (7.0 - C_SCH) + 0.5)  # +0.5: truncation -> round

_CACHE = {}


def _emit(ctx, tc, io):
    import concourse.bass as bass
    import concourse.mybir as mybir

    nc = tc.nc
    f32 = mybir.dt.float32
    bf16 = mybir.dt.bfloat16
    fp16 = mybir.dt.float16
    fp8 = mybir.dt.float8e4
    i8 = mybir.dt.int8
    EXP = mybir.ActivationFunctionType.Exp
    MUL = mybir.AluOpType.mult
    ADD = mybir.AluOpType.add
    DR = mybir.MatmulPerfMode.DoubleRow

    const = ctx.enter_context(tc.tile_pool(name="const", bufs=1))
    xin = ctx.enter_context(tc.tile_pool(name="xin", bufs=1))
    kqp = ctx.enter_context(tc.tile_pool(name="kqp", bufs=4))
    vtp = ctx.enter_context(tc.tile_pool(name="vtp", bufs=4))
    ptp = ctx.enter_context(tc.tile_pool(name="ptp", bufs=3))
    pxe_p = ctx.enter_context(tc.tile_pool(name="pxe", bufs=4))
    xhp = ctx.enter_context(tc.tile_pool(name="xhp", bufs=8))
    work = ctx.enter_context(tc.tile_pool(name="work", bufs=2))
    outp = ctx.enter_context(tc.tile_pool(name="outp", bufs=3))
    psA = ctx.enter_context(tc.tile_pool(name="psA", bufs=3, space="PSUM"))
    psX = ctx.enter_context(tc.tile_pool(name="psX", bufs=2, space="PSUM"))
    dpool = ctx.enter_context(tc.tile_pool(name="dpool", bufs=2, space="DRAM"))

    # ---- constants (gpsimd memsets run during the engine-preamble window)
    wu_a = const.tile([128, 128], bf16, tag="wu_a")
    nc.gpsimd.memset(wu_a, 0.0)
    wu_b = const.tile([128, 512], bf16, tag="wu_b")
    nc.gpsimd.memset(wu_b, 0.0)
    junk = const.tile([128, 2], f32, tag="junk")

    # ---- weights + xq on sync; xk/xv on scalar (xv w0 on gpsimd)
    x_sb = {}
    for name in ("xq", "xk", "xv"):
        x_sb[name] = xin.tile([128, 2, 2048], bf16, tag=name, name=name)
    srcs = {n: io[n].rearrange("(c p) n -> p c n", p=128) for n in ("xq", "xk", "xv")}

    wqkv = const.tile([128, 2, 384], bf16, tag="wqkv")
    nc.sync.dma_start(wqkv, io["wqkv"].rearrange("(c p) o -> p c o", p=128))
    nc.sync.dma_start(x_sb["xk"][:, 1:2, 0:512], srcs["xk"][:, 1:2, 0:512])
    nc.sync.dma_start(x_sb["xq"][:, :, 0:512], srcs["xq"][:, :, 0:512])
    bq_sb = const.tile([128, 1], f32, tag="bq")
    nc.sync.dma_start(bq_sb, io["bq"])
    wm_sb = const.tile([64, 2, 256], bf16, tag="wm")
    nc.sync.dma_start(wm_sb, io["wm"].rearrange("(t o) c -> o t c", t=2))
    nc.sync.dma_start(x_sb["xq"][:, :, 512:2048], srcs["xq"][:, :, 512:2048])

    nc.scalar.dma_start(x_sb["xk"][:, 0:1, 0:512], srcs["xk"][:, 0:1, 0:512])

    # ---- k/q bf16; vt fp8 with ones column
    k_w, q_w, vt_w = [], [], []
    for w in range(NW):
        k_w.append(kqp.tile([128, 512], bf16, tag="kw", name=f"kw{w}"))
        q_w.append(kqp.tile([128, 512], bf16, tag="qw", name=f"qw{w}"))
        vt = vtp.tile([128, 4, 2, 72], fp8, tag="vt", name=f"vt{w}")
        nc.gpsimd.memset(vt[:, :, :, 64:65], 1.0)
        vt_w.append(vt)

    nc.gpsimd.dma_start(x_sb["xv"][:, :, 0:512], srcs["xv"][:, :, 0:512])

    # PE warmup across the input-DMA ramp (HAM clock gate release)
    wu_ps = psA.tile([128, 1024], f32, tag="ps", name="wu_ps")
    for _ in range(10):
        nc.tensor.matmul(wu_ps[:, 0:512], lhsT=wu_a, rhs=wu_b, start=True, stop=True)

    # ACT table load for Exp, after the first xk doorbell
    nc.scalar.activation(junk[:, 0:1], wu_a[:, 0:1], EXP)
    nc.scalar.dma_start(x_sb["xk"][:, :, 512:2048], srcs["xk"][:, :, 512:2048])
    nc.scalar.dma_start(x_sb["xv"][:, :, 512:2048], srcs["xv"][:, :, 512:2048])

    # ---- projection emitters ----
    def proj_k(w):
        ps = psA.tile([128, 1024], f32, tag="ps", name=f"psk{w}")
        for c2 in range(2):
            nc.tensor.matmul(
                ps[:, 0:512],
                lhsT=wqkv[:, c2, 128:256],
                rhs=x_sb["xk"][:, c2, 512 * w : 512 * (w + 1)],
                start=(c2 == 0),
                stop=(c2 == 1),
            )
        if w == 0:
            nc.vector.tensor_copy(k_w[w], ps[:, 0:512])
        else:
            nc.scalar.copy(k_w[w], ps[:, 0:512])

    def proj_q(c):
        ps = psA.tile([128, 1024], f32, tag="ps", name=f"psq{c}")
        for c2 in range(2):
            nc.tensor.matmul(
                ps[:, 0:512],
                lhsT=wqkv[:, c2, 0:128],
                rhs=x_sb["xq"][:, c2, 512 * c : 512 * (c + 1)],
                start=(c2 == 0),
                stop=(c2 == 1),
            )
        nc.vector.tensor_scalar_add(q_w[c], ps[:, 0:512], bq_sb)

    def vt_block(w):
        # 8 matmuls into one PSUM tile (cols off*128 + h*64 + d), ONE evac
        vt = vt_w[w]
        ps = psA.tile([128, 1024], f32, tag="ps", name=f"psvt{w}")
        for off in range(4):
            ms = slice(512 * w + 128 * off, 512 * w + 128 * (off + 1))
            pvt = ps[:, 128 * off : 128 * (off + 1)]
            for c2 in range(2):
                nc.tensor.matmul(
                    pvt,
                    lhsT=x_sb["xv"][:, c2, ms],
                    rhs=wqkv[:, c2, 256:384],
                    start=(c2 == 0),
                    stop=(c2 == 1),
                )
        dst = vt[:, :, :, 0:64]
        src = ps[:, 0:512].rearrange("m (o h d) -> m o h d", o=4, h=2)
        if w % 2 == 0:
            nc.scalar.copy(dst, src)
        else:
            nc.vector.tensor_copy(dst, src)

    # ---- software-pipelined attention ----
    pt_t, px_t, xh_t = {}, {}, {}

    def emit_sc(g):
        c, mb = divmod(g, MB)
        w, off = divmod(mb, 4)
        msl = slice(off * 128, (off + 1) * 128)
        sc = psA.tile([128, 1024], f32, tag="ps", name=f"sc{g}")
        for h in range(2):
            nc.tensor.matmul(
                sc[:, 512 * h : 512 * (h + 1)],
                lhsT=k_w[w][64 * h : 64 * (h + 1), msl],
                rhs=q_w[c][64 * h : 64 * (h + 1), :],
                start=True,
                stop=True,
                tile_position=(64 * h, 0),
            )
        p, i = divmod(g, 2)
        if i == 0:
            pt = ptp.tile([128, 2, 1024], fp8, tag="pt", name=f"pt{p}")
            pt_t[p] = pt
        else:
            pt = pt_t[p]
        if g % 16 in (1, 3, 5, 7, 9, 11, 13) or g == G - 1:
            nc.vector.tensor_scalar(pt[:, i, :].bitcast(i8), sc, S8, B8, MUL, ADD)
        else:
            nc.scalar.activation(pt[:, i, :], sc, EXP, scale=ACT_SCALE)

    def emit_xdr(p):
        c, j = divmod(p, 8)
        w = j // 2
        o2 = (2 * j) % 4
        if j == 0:
            px_t[c] = [
                psX.tile([65, 512], f32, tag="px", name=f"px{c}_{h}") for h in range(2)
            ]
        pt = pt_t.pop(p)
        for h in range(2):
            nc.tensor.matmul(
                px_t[c][h],
                lhsT=vt_w[w][:, o2 : o2 + 2, h, 0:65],
                rhs=pt[:, :, 512 * h : 512 * (h + 1)],
                start=(j == 0),
                stop=(j == 7),
                perf_mode=DR,
                skip_group_check=True,
            )

    def emit_evacs(c):
        px = px_t.pop(c)
        pxe = []
        for h in range(2):
            e = pxe_p.tile([65, 512], f32, tag="pxe", name=f"pxe{c}_{h}")
            if h == 0:
                nc.scalar.copy(e, px[h])
            else:
                nc.vector.tensor_copy(e, px[h])
            pxe.append(e)
        return pxe

    def chunk_tail_rest(c, pxe):
        # 1/sums via [128, 8] reshape, DRAM bounce, partition-broadcast read
        s128 = work.tile([128, 8], f32, tag="s128", name=f"s128_{c}")
        for h in range(2):
            nc.sync.dma_start(s128[64 * h : 64 * (h + 1), :], pxe[h][64:65, :])
        r128 = work.tile([128, 8], f32, tag="r128", name=f"r128_{c}")
        nc.vector.reciprocal(r128, s128)
        r_dram = dpool.tile([1, 1024], f32, tag="r_dram", name=f"r_dram{c}")
        nc.sync.dma_start(r_dram.rearrange("1 (p f) -> p f", p=128), r128)
        r_bc = work.tile([64, 2, 512], f32, tag="r_bc", name=f"r_bc{c}")
        for h in range(2):
            r_src = bass.AP(
                tensor=r_dram.tensor,
                offset=r_dram.offset + h * 512,
                ap=[[0, 64], [1, 512]],
            )
            nc.sync.dma_start(r_bc[:, h, :], r_src)
        for h in range(2):
            xh = xhp.tile([64, 512], bf16, tag="xh", name=f"xh{c}_{h}")
            nc.gpsimd.tensor_mul(xh, pxe[h][0:64, :], r_bc[:, h, :])
            xh_t[(c, h)] = xh

    def final_tail(px):
        # last chunk: per-head UNNORMALIZED out-projection partials + the
        # denominator rows go to DRAM; the host divides and sums. Removes the
        # whole reciprocal/broadcast chain from the critical tail.
        xu = []
        for h in range(2):
            e = xhp.tile([65, 512], fp16, tag="xu", name=f"xu3_{h}")
            if h == 0:
                nc.scalar.copy(e, px[h])
            else:
                nc.vector.tensor_copy(e, px[h])
            xu.append(e)
        ot3 = [
            outp.tile([128, 2, 512], fp16, tag="ot3", name=f"ot3_{h}") for h in range(2)
        ]
        for h in range(2):
            po = psA.tile([128, 1024], f32, tag="ps", name=f"po3_{h}")
            for oc in range(2):
                nc.tensor.matmul(
                    po[:, 512 * oc : 512 * (oc + 1)],
                    lhsT=wm_sb[:, h, 128 * oc : 128 * (oc + 1)],
                    rhs=xu[h][0:64, :],
                    start=True,
                    stop=True,
                )
            src_ = po.rearrange("p (t n) -> p t n", t=2)
            if h == 0:
                nc.scalar.copy(ot3[h], src_)
            else:
                nc.vector.tensor_copy(ot3[h], src_)
        o3 = io["out3"].rearrange("h (t p) n -> h p t n", p=128)
        nc.sync.dma_start(o3[0], ot3[0])
        nc.gpsimd.dma_start(o3[1], ot3[1])
        nc.sync.dma_start(io["den3"][0:1, :], xu[0][64:65, :])
        nc.gpsimd.dma_start(io["den3"][1:2, :], xu[1][64:65, :])

    def out_proj(c):
        ot = outp.tile([128, 2, 512], fp16, tag="ot", name=f"ot{c}")
        po = psA.tile([128, 1024], f32, tag="ps", name=f"po{c}")
        for oc in range(2):
            ocs = slice(128 * oc, 128 * (oc + 1))
            dst = po[:, 512 * oc : 512 * (oc + 1)]
            nc.tensor.matmul(dst, lhsT=wm_sb[:, 0, ocs], rhs=xh_t[(c, 0)], start=True, stop=False)
            nc.tensor.matmul(dst, lhsT=wm_sb[:, 1, ocs], rhs=xh_t[(c, 1)], start=False, stop=True)
        src = po.rearrange("p (t n) -> p t n", t=2)
        if c % 2 == 0:
            nc.scalar.copy(ot, src)
        else:
            nc.vector.tensor_copy(ot, src)
        dst_dram = io["out"].rearrange("(t p) n -> p t n", p=128)[:, :, 512 * c : 512 * (c + 1)]
        eng = nc.sync if c % 2 == 0 else nc.gpsimd
        eng.dma_start(dst_dram, ot)

    def maybe_proj(g):
        c, mb = divmod(g, MB)
        if c == 0:
            if mb in (4, 8, 12):
                proj_k(mb // 4)
            elif mb in (6, 10, 14):
                vt_block((mb - 2) // 4)
        if mb == 0 and c in (1, 2, 3):
            proj_q(c)

    # prelude: window 0 of everything, then pair 0 of scores/exp
    proj_k(0)
    proj_q(0)
    vt_block(0)
    emit_sc(0)
    emit_sc(1)

    for p in range(NP):
        ga, gb = 2 * p + 2, 2 * p + 3
        if p % 8 != 7:
            maybe_proj(ga)
            emit_sc(ga)
            maybe_proj(gb)
            emit_sc(gb)
            emit_xdr(p)
            if p % 8 == 6 and p // 8 >= 1:
                out_proj(p // 8 - 1)
        else:
            c = p // 8
            if ga < G:
                maybe_proj(ga)
                emit_sc(ga)
                emit_xdr(p)
                pxe = emit_evacs(c)
                maybe_proj(gb)
                emit_sc(gb)
                chunk_tail_rest(c, pxe)
            else:
                emit_xdr(p)
                final_tail(px_t.pop(c))


def _build_nc():
    key = "nc"
    if key in _CACHE:
        return _CACHE[key]
    from contextlib import ExitStack

    import concourse.mybir as mybir
    import concourse.tile as tile
    from concourse import bacc

    f32 = mybir.dt.float32
    bf16 = mybir.dt.bfloat16
    fp16 = mybir.dt.float16
    nc = bacc.Bacc("TRN2", target_bir_lowering=False, debug=False, num_devices=8)
    io = {}
    for name, shape, dt_ in (
        ("xq", [256, 2048], bf16),
        ("xk", [256, 2048], bf16),
        ("xv", [256, 2048], bf16),
        ("wqkv", [256, 384], bf16),
        ("bq", [128, 1], f32),
        ("wm", [128, 256], bf16),
    ):
        io[name] = nc.dram_tensor(name, shape, dt_, kind="ExternalInput").ap()
    io["out"] = nc.dram_tensor("out", [256, 2048], fp16, kind="ExternalOutput").ap()
    io["out3"] = nc.dram_tensor("out3", [2, 256, 512], fp16, kind="ExternalOutput").ap()
    io["den3"] = nc.dram_tensor("den3", [2, 512], fp16, kind="ExternalOutput").ap()

    with tile.TileContext(nc) as tc:
        with ExitStack() as ctx:
            _emit(ctx, tc, io)
    nc.compile()
    _CACHE[key] = nc
    return nc


def make_in_maps(query, key, value, wq, bq, wk, bk, wv, bv, wm, bm):
    f = lambda a: np.ascontiguousarray(np.asarray(a), dtype=np.float32)
    fb = lambda a: np.ascontiguousarray(np.asarray(a, dtype=np.float32)).astype(BF)
    query, key, value = f(query), f(key), f(value)
    wq, wk, wv, wm = f(wq), f(wk), f(wv), f(wm)
    bq = f(bq)
    in_maps = []
    for core in range(8):
        b, pair = divmod(core, 2)
        hs = (2 * pair, 2 * pair + 1)
        idx = np.array([d * H + h for h in hs for d in range(DIM)])
        wqkv8 = np.concatenate([wq[idx].T, wk[idx].T, wv[idx].T], axis=1)
        wmh = np.concatenate([wm[:, idx[:64]].T, wm[:, idx[64:]].T], axis=0)
        m = {
            "xq": fb(query[b]),
            "xk": fb(key[b]),
            "xv": fb(value[b]),
            "wqkv": fb(wqkv8),
            "bq": f(bq[idx].reshape(128, 1)),
            "wm": fb(wmh),
        }
        in_maps.append(m)
    return in_maps


def run(in_maps, trace=False, **kw):
    from concourse import bass_utils

    nc = _build_nc()
    return bass_utils.run_bass_kernel_spmd(
        nc, in_maps, core_ids=list(range(8)), trace=trace, **kw
    )


def gather(results, wm, bv, bm):
    wm = np.asarray(wm, dtype=np.float32)
    bv = np.asarray(bv, dtype=np.float32)
    bm = np.asarray(bm, dtype=np.float32)
    corr = bm + wm @ bv
    outs = []
    for r in results:
        o = np.asarray(r["out"], dtype=np.float32)
        o3 = np.asarray(r["out3"], dtype=np.float32)
        den = np.asarray(r["den3"], dtype=np.float32)
        o[:, 3 * 512 :] = (o3 / den[:, None, :]).sum(axis=0)
        outs.append(o)
    return np.stack([outs[2 * b] + outs[2 * b + 1] + corr[:, None] for b in range(B)])


def kernel(query, key, value, wq, bq, wk, bk, wv, bv, wm, bm):
    in_maps = make_in_maps(query, key, value, wq, bq, wk, bk, wv, bv, wm, bm)
    res = run(in_maps)
    return gather(res.results, wm, bv, bm)
